# revision 33
# baseline (speedup 1.0000x reference)
"""ASTRAMoE Trainium2 kernel: gate MLP + top-2-of-4 softmax gating + dense
expert ensemble + Dirichlet alpha head, data-parallel over 8 NeuronCores.

Layout: feature-major on chip (features on SBUF partitions, tokens on the free
axis).  x tiles are PE-transposed once per macro-tile and reused by all six
first-layer matmuls (gate, alpha, 4 experts).  Gate logits are additionally
produced token-major (tokens on partitions) by small N=4 matmuls so that the
top-2 selection + softmax run as cheap per-partition Max8/free-axis ops.  The
gate-weighted expert combine is folded into the PE accumulation: h_e is scaled
by gw_e, then all expert second-layer matmuls (plus a be2 @ gw bias matmul)
accumulate into one PSUM tile.  Outputs leave the device feature-major
([20, n]) and are transposed on the host during the gather; gate weights leave
token-major.  Matmuls run as float32r (full-rate PE) by default.
"""

import os
import sys
from contextlib import ExitStack

import numpy as np

for _p in ("/opt/trn_rl_repo",):
    if _p not in sys.path:
        sys.path.insert(0, _p)

import concourse.bass as bass
import concourse.bacc as bacc
import concourse.tile as tile
from concourse import mybir
from concourse.bass_utils import run_bass_kernel_spmd
from concourse.masks import make_identity

N_TOTAL = 262144
D = 256
H = 256
E = 4
C = 10
NCORES = 8
N_CORE = N_TOTAL // NCORES  # 32768
T = 1024                    # tokens per macro-tile
NT = N_CORE // T            # 32
NST = T // 128              # token sub-tiles per macro-tile
TH = T // 2                 # fp32 moving-operand max
FP = mybir.dt.float32
FPR = mybir.dt.float32r
BF = mybir.dt.bfloat16
AF = mybir.ActivationFunctionType
ALU = mybir.AluOpType
NEG = -1e30


def bcast_inner(ap, n):
    """Broadcast an AP along a new innermost (step-0) axis of length n."""
    return bass.AP(tensor=ap.tensor, offset=ap.offset, ap=list(ap.ap) + [[0, n]])


def build_kernel(mm_dt=BF, gate_dt=FP):
    def mdt(ap, dt):
        return ap.bitcast(dt) if dt != FP else ap

    nc = bacc.Bacc("TRN2", debug=False, enable_asserts=False)

    x = nc.dram_tensor("x", [N_CORE, D], FP, kind="ExternalInput").ap()
    Wg1 = nc.dram_tensor("Wg1", [D, D], FP, kind="ExternalInput").ap()
    bg1 = nc.dram_tensor("bg1", [D], FP, kind="ExternalInput").ap()
    Wg2 = nc.dram_tensor("Wg2", [D, E], FP, kind="ExternalInput").ap()
    bg2 = nc.dram_tensor("bg2", [E], FP, kind="ExternalInput").ap()
    We1 = nc.dram_tensor("We1", [E, D, H], FP, kind="ExternalInput").ap()
    be1 = nc.dram_tensor("be1", [E, H], FP, kind="ExternalInput").ap()
    We2 = nc.dram_tensor("We2", [E, H, C], FP, kind="ExternalInput").ap()
    be2 = nc.dram_tensor("be2", [E, C], FP, kind="ExternalInput").ap()
    Wa1 = nc.dram_tensor("Wa1", [D, H], FP, kind="ExternalInput").ap()
    ba1 = nc.dram_tensor("ba1", [H], FP, kind="ExternalInput").ap()
    Wa2 = nc.dram_tensor("Wa2", [H, C], FP, kind="ExternalInput").ap()
    ba2 = nc.dram_tensor("ba2", [C], FP, kind="ExternalInput").ap()

    # outT rows: 0..9 logits^T, 10..19 alpha^T ; gate weights token-major
    outT = nc.dram_tensor("outT", [2 * C, N_CORE], FP, kind="ExternalOutput").ap()
    gw_outT = nc.dram_tensor("gw_outT", [E, N_CORE], FP, kind="ExternalOutput").ap()
    load_out = nc.dram_tensor("load_out", [1, NST * E], FP, kind="ExternalOutput").ap()

    with tile.TileContext(nc) as tc, ExitStack() as ctx:
        singles = ctx.enter_context(tc.tile_pool(name="singles", bufs=1))
        xn_pool = ctx.enter_context(tc.tile_pool(name="xn", bufs=2))
        xt_pool = ctx.enter_context(tc.tile_pool(name="xt", bufs=2))
        h_pool = ctx.enter_context(tc.tile_pool(name="h", bufs=2))
        g_pool = ctx.enter_context(tc.tile_pool(name="g", bufs=2))
        out_pool = ctx.enter_context(tc.tile_pool(name="out", bufs=2))
        gwd_pool = ctx.enter_context(tc.tile_pool(name="gwd", bufs=2, space="DRAM"))
        tp_psum = ctx.enter_context(tc.tile_pool(name="tp_ps", bufs=2, space="PSUM"))
        l1_psum = ctx.enter_context(tc.tile_pool(name="l1_ps", bufs=2, space="PSUM"))
        l2_psum = ctx.enter_context(tc.tile_pool(name="l2_ps", bufs=2, space="PSUM"))

        ident = singles.tile([128, 128], FP)
        make_identity(nc, ident)

        # --- stationary weights, feature-major lhsT tiles -------------------
        wg1_sb = singles.tile([128, 2, D], FP)
        nc.sync.dma_start(out=wg1_sb, in_=Wg1.rearrange("(kt p) h -> p kt h", p=128))
        wa1_sb = singles.tile([128, 2, H], FP)
        nc.sync.dma_start(out=wa1_sb, in_=Wa1.rearrange("(kt p) h -> p kt h", p=128))
        we1_sb = singles.tile([128, E, 2, H], FP)
        nc.sync.dma_start(out=we1_sb, in_=We1.rearrange("e (kt p) h -> p e kt h", p=128))
        wg2_sb = singles.tile([128, 2, E], FP)
        nc.sync.dma_start(out=wg2_sb, in_=Wg2.rearrange("(kt p) c -> p kt c", p=128))
        wa2_sb = singles.tile([128, 2, C], FP)
        nc.sync.dma_start(out=wa2_sb, in_=Wa2.rearrange("(kt p) c -> p kt c", p=128))
        we2_sb = singles.tile([128, E, 2, C], FP)
        nc.sync.dma_start(out=we2_sb, in_=We2.rearrange("e (kt p) c -> p e kt c", p=128))
        be2_sb = singles.tile([E, C], FP)
        nc.sync.dma_start(out=be2_sb, in_=be2)
        be2_r = singles.tile([E, C], FPR)
        nc.vector.tensor_copy(out=be2_r, in_=be2_sb)

        bg1_sb = singles.tile([128, 2], FP)
        nc.sync.dma_start(out=bg1_sb, in_=bg1.rearrange("(t p) -> p t", p=128))
        ba1_sb = singles.tile([128, 2], FP)
        nc.sync.dma_start(out=ba1_sb, in_=ba1.rearrange("(t p) -> p t", p=128))
        be1_sb = singles.tile([128, E, 2], FP)
        nc.sync.dma_start(out=be1_sb, in_=be1.rearrange("e (t p) -> p e t", p=128))
        ba2_sb = singles.tile([C, 1], FP)
        nc.sync.dma_start(out=ba2_sb, in_=ba2.rearrange("(c o) -> c o", o=1))

        # bg2 broadcast across all partitions (token-major bias)
        bg2_row = singles.tile([1, E], FP)
        nc.sync.dma_start(out=bg2_row, in_=bg2.rearrange("(o e) -> o e", o=1))
        bg2_tok = singles.tile([128, E], FP)
        nc.gpsimd.partition_broadcast(bg2_tok, bg2_row)

        # rounded float32r copies of weights feeding reduced-precision matmuls
        if mm_dt != FP:
            wa1_r = singles.tile([128, 2, H], mm_dt)
            nc.vector.tensor_copy(out=wa1_r, in_=wa1_sb)
            we1_r = singles.tile([128, E, 2, H], mm_dt)
            nc.vector.tensor_copy(out=we1_r, in_=we1_sb)
            wa2_r = singles.tile([128, 2, C], mm_dt)
            nc.vector.tensor_copy(out=wa2_r, in_=wa2_sb)
            we2_r = singles.tile([128, E, 2, C], mm_dt)
            nc.vector.tensor_copy(out=we2_r, in_=we2_sb)
        else:
            wa1_r, we1_r, wa2_r, we2_r = wa1_sb, we1_sb, wa2_sb, we2_sb
        if gate_dt == FPR:
            wg1_g = singles.tile([128, 2, D], FPR)
            nc.vector.tensor_copy(out=wg1_g, in_=wg1_sb)
            wg2_g = singles.tile([128, 2, E], FPR)
            nc.vector.tensor_copy(out=wg2_g, in_=wg2_sb)
        else:
            wg1_g, wg2_g = wg1_sb, wg2_sb

        acc_load = singles.tile([128, NST, E], FP)
        nc.vector.memset(acc_load, 0.0)

        zd_pool = ctx.enter_context(tc.tile_pool(name="zd", bufs=1, space="DRAM"))
        z_dram = zd_pool.tile([C, N_CORE], FP, tag="zd")

        prev = None  # deferred expert-L2 state from the previous tile
        for it in range(NT + 1):
            if it < NT:
                # ---- load + transpose x -----------------------------------
                xn = xn_pool.tile([128, NST, D], FP, tag="xn")
                nc.sync.dma_start(
                    out=xn, in_=x[it * T:(it + 1) * T, :].rearrange("(tt p) d -> p tt d", p=128)
                )
                xt = xt_pool.tile([128, 2, T], FP, tag="xt")
                for dt in range(2):
                    for tt in range(NST):
                        ps = tp_psum.tile([128, 128], FP, tag="tp")
                        nc.tensor.transpose(ps, xn[:, tt, dt * 128:(dt + 1) * 128], ident)
                        if tt % 2 == 0:
                            nc.vector.tensor_copy(out=xt[:, dt, tt * 128:(tt + 1) * 128], in_=ps)
                        else:
                            nc.scalar.copy(out=xt[:, dt, tt * 128:(tt + 1) * 128], in_=ps)
                xt_r = xt_pool.tile([128, 2, T], mm_dt, tag="xtr")
                nc.vector.tensor_copy(out=xt_r, in_=xt)

                # ---- gate MLP (feature-major L1, token-major L2) ----------
                g1 = h_pool.tile([128, 2, T], FP, tag="hg")
                for ht in range(2):
                    ps_g = l1_psum.tile([128, T], FP, tag="l1")
                    for th in range(2):
                        for kt in range(2):
                            nc.tensor.matmul(
                                ps_g[:, th * TH:(th + 1) * TH],
                                wg1_g[:, kt, ht * 128:(ht + 1) * 128],
                                xt[:, kt, th * TH:(th + 1) * TH],
                                start=(kt == 0), stop=(kt == 1),
                            )
                    nc.scalar.activation(
                        out=g1[:, ht, :], in_=ps_g, func=AF.Gelu,
                        bias=bg1_sb[:, ht:ht + 1],
                    )
                ps_lt = tp_psum.tile([128, NST, E], FP, tag="tp")
                for st in range(NST):
                    for kt in range(2):
                        nc.tensor.matmul(
                            ps_lt[:, st, :],
                            g1[:, kt, st * 128:(st + 1) * 128],
                            wg2_g[:, kt, :],
                            start=(kt == 0), stop=(kt == 1),
                        )

            # ---- deferred: previous tile's expert L2 + logits out ---------
            if prev is not None:
                p_it, p_he, p_gw_fm_r = prev
                out_sb = out_pool.tile([C, T], FP, tag="out")
                for th in range(2):
                    ps_lg = l2_psum.tile([C, TH], FP, tag="l2")
                    for e in range(E):
                        for kt in range(2):
                            nc.tensor.matmul(
                                ps_lg, we2_r[:, e, kt, :],
                                p_he[e][:, kt, th * TH:(th + 1) * TH],
                                start=(e == 0 and kt == 0), stop=False,
                                skip_group_check=True,
                            )
                    nc.tensor.matmul(ps_lg, be2_r,
                                     p_gw_fm_r[:, th * TH:(th + 1) * TH],
                                     start=False, stop=True,
                                     skip_group_check=True)
                    nc.vector.tensor_copy(out=out_sb[:, th * TH:(th + 1) * TH],
                                          in_=ps_lg)
                nc.sync.dma_start(out=outT[0:C, p_it * T:(p_it + 1) * T], in_=out_sb)
                prev = None

            if it >= NT:
                break

            # ---- top-2-of-4 mask + softmax (token-major) ------------------
            ltok = g_pool.tile([128, NST, 2 * E], FP, tag="ltok")
            top8 = g_pool.tile([128, NST, 8], FP, tag="top8")
            gwr = g_pool.tile([128, NST, E], FP, tag="gwr")
            msk = g_pool.tile([128, NST, E], FP, tag="msk")
            nc.vector.memset(ltok[:, :, E:], NEG)
            nc.vector.tensor_add(
                ltok[:, :, 0:E], ps_lt,
                bass.AP(tensor=bg2_tok.tensor, offset=bg2_tok.offset,
                        ap=[bg2_tok.ap[0], [0, NST], [1, E]]),
            )
            for st in range(NST):
                nc.vector.max(out=top8[:, st, :], in_=ltok[:, st, :])
            x3b = bass.AP(tensor=top8.tensor, offset=top8.offset + 2,
                          ap=[top8.ap[0], [8, NST], [0, E]])
            nc.vector.tensor_tensor(out=msk, in0=ltok[:, :, 0:E], in1=x3b,
                                    op=ALU.is_gt)
            nc.vector.tensor_tensor(out=gwr, in0=ltok[:, :, 0:E], in1=msk,
                                    op=ALU.mult)
            # exp via tanh (stays in the gelu table set): e^m = (1+t)/(1-t)
            th = g_pool.tile([128, NST, E], FP, tag="th")
            nc.scalar.activation(out=th, in_=gwr, func=AF.Tanh, scale=0.5)
            nm = g_pool.tile([128, NST, E], FP, tag="nm")
            nc.vector.tensor_scalar_add(out=nm, in0=th, scalar1=1.0)
            dn = g_pool.tile([128, NST, E], FP, tag="dn")
            nc.vector.tensor_scalar(out=dn, in0=th, scalar1=-1.0, scalar2=1.0,
                                    op0=ALU.mult, op1=ALU.add)
            rdn = g_pool.tile([128, NST, E], FP, tag="rdn")
            nc.vector.reciprocal(out=rdn, in_=dn)
            egw = g_pool.tile([128, NST, E], FP, tag="egw")
            nc.vector.tensor_tensor(out=egw, in0=nm, in1=rdn, op=ALU.mult)
            ssum = g_pool.tile([128, NST], FP, tag="ssum")
            nc.vector.tensor_reduce(out=ssum, in_=egw, op=ALU.add,
                                    axis=mybir.AxisListType.X)
            rcp = g_pool.tile([128, NST], FP, tag="rcp")
            nc.vector.reciprocal(out=rcp, in_=ssum)
            gw_all = g_pool.tile([128, NST, E], FP, tag="gw")
            nc.vector.tensor_tensor(out=gw_all, in0=egw, in1=bcast_inner(rcp, E),
                                    op=ALU.mult)
            nc.vector.tensor_add(acc_load, acc_load, gw_all)
            # one PE transpose to (st,e)-major, then contiguous DRAM bounce
            ps_gwt = tp_psum.tile([NST * E, 128], FP, tag="tp")
            nc.tensor.transpose(ps_gwt, gw_all.rearrange("p a b -> p (a b)"), ident)
            gwt_sb = g_pool.tile([NST * E, 128], FP, tag="gwt")
            nc.vector.tensor_copy(out=gwt_sb, in_=ps_gwt)
            gwt_bf = g_pool.tile([NST * E, 128], mm_dt, tag="gwtbf")
            nc.vector.tensor_copy(out=gwt_bf, in_=gwt_sb)
            gw_dram = gwd_pool.tile([E, T], FP, tag="gwd")
            nc.sync.dma_start(
                out=bass.AP(tensor=gw_dram.tensor, offset=gw_dram.offset,
                            ap=[[128, NST], [T, E], [1, 128]]),
                in_=gwt_sb,
            )
            gw_dram_bf = gwd_pool.tile([E, T], mm_dt, tag="gwdbf")
            nc.sync.dma_start(
                out=bass.AP(tensor=gw_dram_bf.tensor, offset=gw_dram_bf.offset,
                            ap=[[128, NST], [T, E], [1, 128]]),
                in_=gwt_bf,
            )
            nc.sync.dma_start(
                out=gw_outT[:, it * T:(it + 1) * T],
                in_=bass.AP(tensor=gw_dram.tensor, offset=gw_dram.offset,
                            ap=[[T, E], [1, T]]),
            )
            gw_fm = g_pool.tile([E, T], FP, tag="gwfm")
            nc.sync.dma_start(
                out=gw_fm,
                in_=bass.AP(tensor=gw_dram.tensor, offset=gw_dram.offset,
                            ap=[[T, E], [1, T]]),
            )
            gw_fm_r = g_pool.tile([E, T], FPR, tag="gwfmr")
            nc.vector.tensor_copy(out=gw_fm_r, in_=gw_fm)
            gwb = g_pool.tile([128, E, T], mm_dt, tag="gwb")
            for e in range(E):
                nc.sync.dma_start(
                    out=gwb[:, e, :],
                    in_=bass.AP(tensor=gw_dram_bf.tensor,
                                offset=gw_dram_bf.offset + e * T,
                                ap=[[0, 128], [1, T]]),
                )

            # ---- alpha head -------------------------------------------
            a1 = h_pool.tile([128, 2, T], mm_dt, tag="ha")
            for ht in range(2):
                ps_a = l1_psum.tile([128, T], FP, tag="l1")
                for th in range(2):
                    for kt in range(2):
                        nc.tensor.matmul(
                            ps_a[:, th * TH:(th + 1) * TH],
                            wa1_r[:, kt, ht * 128:(ht + 1) * 128],
                            xt_r[:, kt, th * TH:(th + 1) * TH],
                            start=(kt == 0), stop=(kt == 1),
                        )
                nc.scalar.activation(
                    out=a1[:, ht, :], in_=ps_a, func=AF.Gelu,
                    bias=ba1_sb[:, ht:ht + 1],
                )
            z_sb = out_pool.tile([C, T], FP, tag="z")
            for th in range(2):
                ps_z = l2_psum.tile([C, TH], FP, tag="l2")
                for kt in range(2):
                    nc.tensor.matmul(
                        ps_z, wa2_r[:, kt, :],
                        a1[:, kt, th * TH:(th + 1) * TH],
                        start=(kt == 0), stop=(kt == 1),
                    )
                nc.vector.tensor_scalar_add(
                    out=z_sb[:, th * TH:(th + 1) * TH], in0=ps_z,
                    scalar1=ba2_sb)
            nc.sync.dma_start(out=z_dram[:, it * T:(it + 1) * T], in_=z_sb)

            # ---- experts: h_e = gelu(x@We1[e]+be1[e]) * gw_e --------------
            he_list = []
            for e in range(E):
                he = h_pool.tile([128, 2, T], mm_dt, tag=f"he{e}")
                for ht in range(2):
                    ps_e = l1_psum.tile([128, T], FP, tag="l1")
                    for th in range(2):
                        for kt in range(2):
                            nc.tensor.matmul(
                                ps_e[:, th * TH:(th + 1) * TH],
                                we1_r[:, e, kt, ht * 128:(ht + 1) * 128],
                                xt_r[:, kt, th * TH:(th + 1) * TH],
                                start=(kt == 0), stop=(kt == 1),
                            )
                    nc.scalar.activation(
                        out=he[:, ht, :], in_=ps_e, func=AF.Gelu,
                        bias=be1_sb[:, e, ht:ht + 1],
                    )
                    nc.vector.tensor_tensor(
                        out=he[:, ht, :], in0=he[:, ht, :], in1=gwb[:, e, :],
                        op=ALU.mult,
                    )
                he_list.append(he)
            prev = (it, he_list, gw_fm_r)

        # ---- pass B: alpha = softplus(z) + 1e-6 over the flat z block ------
        spb = ctx.enter_context(tc.tile_pool(name="spb", bufs=1))
        ZF = C * N_CORE // 128          # 2560 elements per partition
        zc = spb.tile([128, ZF], FP)
        nc.sync.dma_start(
            out=zc,
            in_=bass.AP(tensor=z_dram.tensor, offset=z_dram.offset,
                        ap=[[ZF, 128], [1, ZF]]),
        )
        ec = spb.tile([128, ZF], FP)
        nc.scalar.activation(out=ec, in_=zc, func=AF.Exp)
        ac = spb.tile([128, ZF], FP)
        nc.scalar.activation(out=ac, in_=ec, func=AF.Ln, bias=1.0)
        nc.vector.tensor_scalar_add(out=ac, in0=ac, scalar1=1e-6)
        nc.sync.dma_start(
            out=bass.AP(tensor=outT.tensor, offset=outT.offset + C * N_CORE,
                        ap=[[ZF, 128], [1, ZF]]),
            in_=ac,
        )

        # ---- load: sum over token partitions via ones-matmul; host finishes
        ones_col = singles.tile([128, 1], FP)
        nc.vector.memset(ones_col, 1.0)
        ps_load = l2_psum.tile([1, NST * E], FP, tag="l2")
        nc.tensor.matmul(ps_load, ones_col,
                         acc_load.rearrange("p a b -> p (a b)"),
                         start=True, stop=True)
        load_sb = singles.tile([1, NST * E], FP)
        nc.vector.tensor_copy(out=load_sb, in_=ps_load)
        nc.sync.dma_start(out=load_out, in_=load_sb)

    nc.compile()
    return nc


_NC_CACHE = None


def kernel(**inputs):
    global _NC_CACHE
    if _NC_CACHE is None:
        if os.environ.get("BASS_MOE_FP32"):
            _NC_CACHE = build_kernel(mm_dt=FP, gate_dt=FP)
        else:
            _NC_CACHE = build_kernel()
    nc = _NC_CACHE

    f32 = lambda a: np.ascontiguousarray(np.asarray(a, dtype=np.float32))
    x = f32(inputs["x"])
    weights = {k: f32(inputs[k]) for k in
               ("Wg1", "bg1", "Wg2", "bg2", "We1", "be1", "We2", "be2",
                "Wa1", "ba1", "Wa2", "ba2")}

    in_maps = []
    for c in range(NCORES):
        m = {"x": f32(x[c * N_CORE:(c + 1) * N_CORE])}
        m.update(weights)
        in_maps.append(m)

    res = run_bass_kernel_spmd(nc, in_maps, core_ids=list(range(NCORES)),
                               trace=False)

    outT = np.concatenate([r["outT"] for r in res.results], axis=1)   # [20, N]
    gw = np.concatenate([r["gw_outT"] for r in res.results], axis=1).T  # [N, 4]
    load = np.sum([r["load_out"][0].reshape(NST, E).sum(0) for r in res.results],
                  axis=0, dtype=np.float32).astype(np.float32)
    big = np.ascontiguousarray(outT.T)
    logits = big[:, 0:C]
    alpha = big[:, C:2 * C]
    return logits, alpha, np.ascontiguousarray(gw), load


# revision 34
# speedup vs baseline: 1.3285x; 1.3285x over previous
"""ASTRAMoE Trainium2 kernel: gate MLP + top-2-of-4 softmax gating + dense
expert ensemble + Dirichlet alpha head, data-parallel over 8 NeuronCores.

Layout: feature-major on chip (features on SBUF partitions, tokens on the free
axis).  x tiles are PE-transposed once per macro-tile and reused by all six
first-layer matmuls (gate, alpha, 4 experts).  Gate logits are additionally
produced token-major (tokens on partitions) by small N=4 matmuls so that the
top-2 selection + softmax run as cheap per-partition Max8/free-axis ops.  The
gate-weighted expert combine is folded into the PE accumulation: h_e is scaled
by gw_e, then all expert second-layer matmuls (plus a be2 @ gw bias matmul)
accumulate into one PSUM tile.  Outputs leave the device feature-major
([20, n]) and are transposed on the host during the gather; gate weights leave
token-major.  Matmuls run as float32r (full-rate PE) by default.
"""

import os
import sys
from contextlib import ExitStack

import numpy as np

for _p in ("/opt/trn_rl_repo",):
    if _p not in sys.path:
        sys.path.insert(0, _p)

import concourse.bass as bass
import concourse.bacc as bacc
import concourse.tile as tile
from concourse import mybir
from concourse.bass_utils import run_bass_kernel_spmd
from concourse.masks import make_identity

N_TOTAL = 262144
D = 256
H = 256
E = 4
C = 10
NCORES = 8
N_CORE = N_TOTAL // NCORES  # 32768
T = 1024                    # tokens per macro-tile
NT = N_CORE // T            # 32
NST = T // 128              # token sub-tiles per macro-tile
TH = T // 2                 # fp32 moving-operand max
FP = mybir.dt.float32
FPR = mybir.dt.float32r
BF = mybir.dt.bfloat16
AF = mybir.ActivationFunctionType
ALU = mybir.AluOpType
NEG = -1e30


def bcast_inner(ap, n):
    """Broadcast an AP along a new innermost (step-0) axis of length n."""
    return bass.AP(tensor=ap.tensor, offset=ap.offset, ap=list(ap.ap) + [[0, n]])


def build_kernel(mm_dt=BF, gate_dt=FP):
    def mdt(ap, dt):
        return ap.bitcast(dt) if dt != FP else ap

    nc = bacc.Bacc("TRN2", debug=False, enable_asserts=False)

    x = nc.dram_tensor("x", [N_CORE, D], FP, kind="ExternalInput").ap()
    Wg1 = nc.dram_tensor("Wg1", [D, D], FP, kind="ExternalInput").ap()
    bg1 = nc.dram_tensor("bg1", [D], FP, kind="ExternalInput").ap()
    Wg2 = nc.dram_tensor("Wg2", [D, E], FP, kind="ExternalInput").ap()
    bg2 = nc.dram_tensor("bg2", [E], FP, kind="ExternalInput").ap()
    We1 = nc.dram_tensor("We1", [E, D, H], FP, kind="ExternalInput").ap()
    be1 = nc.dram_tensor("be1", [E, H], FP, kind="ExternalInput").ap()
    We2 = nc.dram_tensor("We2", [E, H, C], FP, kind="ExternalInput").ap()
    be2 = nc.dram_tensor("be2", [E, C], FP, kind="ExternalInput").ap()
    Wa1 = nc.dram_tensor("Wa1", [D, H], FP, kind="ExternalInput").ap()
    ba1 = nc.dram_tensor("ba1", [H], FP, kind="ExternalInput").ap()
    Wa2 = nc.dram_tensor("Wa2", [H, C], FP, kind="ExternalInput").ap()
    ba2 = nc.dram_tensor("ba2", [C], FP, kind="ExternalInput").ap()

    # outT rows: 0..9 logits^T, 10..19 alpha^T ; gate weights token-major
    outT = nc.dram_tensor("outT", [2 * C, N_CORE], FP, kind="ExternalOutput").ap()
    gw_outT = nc.dram_tensor("gw_outT", [E, N_CORE], FP, kind="ExternalOutput").ap()
    load_out = nc.dram_tensor("load_out", [1, NST * E], FP, kind="ExternalOutput").ap()

    with tile.TileContext(nc) as tc, ExitStack() as ctx:
        singles = ctx.enter_context(tc.tile_pool(name="singles", bufs=1))
        xn_pool = ctx.enter_context(tc.tile_pool(name="xn", bufs=2))
        xt_pool = ctx.enter_context(tc.tile_pool(name="xt", bufs=2))
        h_pool = ctx.enter_context(tc.tile_pool(name="h", bufs=2))
        g_pool = ctx.enter_context(tc.tile_pool(name="g", bufs=2))
        out_pool = ctx.enter_context(tc.tile_pool(name="out", bufs=2))
        gwd_pool = ctx.enter_context(tc.tile_pool(name="gwd", bufs=2, space="DRAM"))
        tp_psum = ctx.enter_context(tc.tile_pool(name="tp_ps", bufs=2, space="PSUM"))
        l1_psum = ctx.enter_context(tc.tile_pool(name="l1_ps", bufs=2, space="PSUM"))
        l2_psum = ctx.enter_context(tc.tile_pool(name="l2_ps", bufs=2, space="PSUM"))

        ident = singles.tile([128, 128], FP)
        make_identity(nc, ident)

        # --- stationary weights, feature-major lhsT tiles -------------------
        wg1_sb = singles.tile([128, 2, D], FP)
        nc.sync.dma_start(out=wg1_sb, in_=Wg1.rearrange("(kt p) h -> p kt h", p=128))
        wa1_sb = singles.tile([128, 2, H], FP)
        nc.sync.dma_start(out=wa1_sb, in_=Wa1.rearrange("(kt p) h -> p kt h", p=128))
        we1_sb = singles.tile([128, E, 2, H], FP)
        nc.sync.dma_start(out=we1_sb, in_=We1.rearrange("e (kt p) h -> p e kt h", p=128))
        wg2_sb = singles.tile([128, 2, E], FP)
        nc.sync.dma_start(out=wg2_sb, in_=Wg2.rearrange("(kt p) c -> p kt c", p=128))
        wa2_sb = singles.tile([128, 2, C], FP)
        nc.sync.dma_start(out=wa2_sb, in_=Wa2.rearrange("(kt p) c -> p kt c", p=128))
        we2_sb = singles.tile([128, E, 2, C], FP)
        nc.sync.dma_start(out=we2_sb, in_=We2.rearrange("e (kt p) c -> p e kt c", p=128))
        be2_sb = singles.tile([E, C], FP)
        nc.sync.dma_start(out=be2_sb, in_=be2)
        be2_r = singles.tile([E, C], FPR)
        nc.vector.tensor_copy(out=be2_r, in_=be2_sb)

        bg1_sb = singles.tile([128, 2], FP)
        nc.sync.dma_start(out=bg1_sb, in_=bg1.rearrange("(t p) -> p t", p=128))
        ba1_sb = singles.tile([128, 2], FP)
        nc.sync.dma_start(out=ba1_sb, in_=ba1.rearrange("(t p) -> p t", p=128))
        be1_sb = singles.tile([128, E, 2], FP)
        nc.sync.dma_start(out=be1_sb, in_=be1.rearrange("e (t p) -> p e t", p=128))
        ba2_sb = singles.tile([C, 1], FP)
        nc.sync.dma_start(out=ba2_sb, in_=ba2.rearrange("(c o) -> c o", o=1))

        # bg2 broadcast across all partitions (token-major bias)
        bg2_row = singles.tile([1, E], FP)
        nc.sync.dma_start(out=bg2_row, in_=bg2.rearrange("(o e) -> o e", o=1))
        bg2_tok = singles.tile([128, E], FP)
        nc.gpsimd.partition_broadcast(bg2_tok, bg2_row)

        # rounded float32r copies of weights feeding reduced-precision matmuls
        if mm_dt != FP:
            wa1_r = singles.tile([128, 2, H], mm_dt)
            nc.vector.tensor_copy(out=wa1_r, in_=wa1_sb)
            we1_r = singles.tile([128, E, 2, H], mm_dt)
            nc.vector.tensor_copy(out=we1_r, in_=we1_sb)
            wa2_r = singles.tile([128, 2, C], mm_dt)
            nc.vector.tensor_copy(out=wa2_r, in_=wa2_sb)
            we2_r = singles.tile([128, E, 2, C], mm_dt)
            nc.vector.tensor_copy(out=we2_r, in_=we2_sb)
        else:
            wa1_r, we1_r, wa2_r, we2_r = wa1_sb, we1_sb, wa2_sb, we2_sb
        if gate_dt == FPR:
            wg1_g = singles.tile([128, 2, D], FPR)
            nc.vector.tensor_copy(out=wg1_g, in_=wg1_sb)
            wg2_g = singles.tile([128, 2, E], FPR)
            nc.vector.tensor_copy(out=wg2_g, in_=wg2_sb)
        else:
            wg1_g, wg2_g = wg1_sb, wg2_sb

        acc_load = singles.tile([128, NST, E], FP)
        nc.vector.memset(acc_load, 0.0)

        zd_pool = ctx.enter_context(tc.tile_pool(name="zd", bufs=1, space="DRAM"))
        z_dram = zd_pool.tile([C, N_CORE], FP, tag="zd")

        prev = None  # deferred expert-L2 state from the previous tile
        for it in range(NT + 1):
            if it < NT:
                # ---- load + transpose x -----------------------------------
                xn = xn_pool.tile([128, NST, D], FP, tag="xn")
                nc.sync.dma_start(
                    out=xn, in_=x[it * T:(it + 1) * T, :].rearrange("(tt p) d -> p tt d", p=128)
                )
                xt = xt_pool.tile([128, 2, T], FP, tag="xt")
                for dt in range(2):
                    for tg in range(NST // 4):
                        ps4 = tp_psum.tile([128, 512], FP, tag="tp")
                        for tj in range(4):
                            tt = tg * 4 + tj
                            nc.tensor.transpose(ps4[:, tj * 128:(tj + 1) * 128],
                                                xn[:, tt, dt * 128:(dt + 1) * 128], ident)
                        nc.vector.tensor_copy(
                            out=xt[:, dt, tg * 512:(tg + 1) * 512], in_=ps4)
                xt_r = xt_pool.tile([128, 2, T], mm_dt, tag="xtr")
                nc.vector.tensor_copy(out=xt_r, in_=xt)

                # ---- gate MLP (feature-major L1, token-major L2) ----------
                g1 = h_pool.tile([128, 2, T], FP, tag="hg")
                for ht in range(2):
                    ps_g = l1_psum.tile([128, T], FP, tag="l1")
                    for th in range(2):
                        for kt in range(2):
                            nc.tensor.matmul(
                                ps_g[:, th * TH:(th + 1) * TH],
                                wg1_g[:, kt, ht * 128:(ht + 1) * 128],
                                xt[:, kt, th * TH:(th + 1) * TH],
                                start=(kt == 0), stop=(kt == 1),
                            )
                    nc.scalar.activation(
                        out=g1[:, ht, :], in_=ps_g, func=AF.Gelu,
                        bias=bg1_sb[:, ht:ht + 1],
                    )
                ps_lt = tp_psum.tile([128, NST, E], FP, tag="tp")
                for st in range(NST):
                    for kt in range(2):
                        nc.tensor.matmul(
                            ps_lt[:, st, :],
                            g1[:, kt, st * 128:(st + 1) * 128],
                            wg2_g[:, kt, :],
                            start=(kt == 0), stop=(kt == 1),
                        )

            # ---- deferred: previous tile's expert L2 + logits out ---------
            if prev is not None:
                p_it, p_he, p_gw_fm_r = prev
                out_sb = out_pool.tile([C, T], FP, tag="out")
                for th in range(2):
                    ps_lg = l2_psum.tile([C, TH], FP, tag="l2")
                    for e in range(E):
                        for kt in range(2):
                            nc.tensor.matmul(
                                ps_lg, we2_r[:, e, kt, :],
                                p_he[e][:, kt, th * TH:(th + 1) * TH],
                                start=(e == 0 and kt == 0), stop=False,
                                skip_group_check=True,
                            )
                    nc.tensor.matmul(ps_lg, be2_r,
                                     p_gw_fm_r[:, th * TH:(th + 1) * TH],
                                     start=False, stop=True,
                                     skip_group_check=True)
                    nc.vector.tensor_copy(out=out_sb[:, th * TH:(th + 1) * TH],
                                          in_=ps_lg)
                nc.sync.dma_start(out=outT[0:C, p_it * T:(p_it + 1) * T], in_=out_sb)
                prev = None

            if it >= NT:
                break

            # ---- top-2-of-4 mask + softmax (token-major) ------------------
            ltok = g_pool.tile([128, NST, 2 * E], FP, tag="ltok")
            top8 = g_pool.tile([128, NST, 8], FP, tag="top8")
            gwr = g_pool.tile([128, NST, E], FP, tag="gwr")
            msk = g_pool.tile([128, NST, E], FP, tag="msk")
            nc.vector.memset(ltok[:, :, E:], NEG)
            nc.vector.tensor_add(
                ltok[:, :, 0:E], ps_lt,
                bass.AP(tensor=bg2_tok.tensor, offset=bg2_tok.offset,
                        ap=[bg2_tok.ap[0], [0, NST], [1, E]]),
            )
            for st in range(NST):
                nc.vector.max(out=top8[:, st, :], in_=ltok[:, st, :])
            x3b = bass.AP(tensor=top8.tensor, offset=top8.offset + 2,
                          ap=[top8.ap[0], [8, NST], [0, E]])
            nc.vector.tensor_tensor(out=msk, in0=ltok[:, :, 0:E], in1=x3b,
                                    op=ALU.is_gt)
            nc.vector.tensor_tensor(out=gwr, in0=ltok[:, :, 0:E], in1=msk,
                                    op=ALU.mult)
            # exp via tanh (stays in the gelu table set): e^m = (1+t)/(1-t)
            th = g_pool.tile([128, NST, E], FP, tag="th")
            nc.scalar.activation(out=th, in_=gwr, func=AF.Tanh, scale=0.5)
            nm = g_pool.tile([128, NST, E], FP, tag="nm")
            nc.vector.tensor_scalar_add(out=nm, in0=th, scalar1=1.0)
            dn = g_pool.tile([128, NST, E], FP, tag="dn")
            nc.vector.tensor_scalar(out=dn, in0=th, scalar1=-1.0, scalar2=1.0,
                                    op0=ALU.mult, op1=ALU.add)
            rdn = g_pool.tile([128, NST, E], FP, tag="rdn")
            nc.vector.reciprocal(out=rdn, in_=dn)
            egw = g_pool.tile([128, NST, E], FP, tag="egw")
            nc.vector.tensor_tensor(out=egw, in0=nm, in1=rdn, op=ALU.mult)
            ssum = g_pool.tile([128, NST], FP, tag="ssum")
            nc.vector.tensor_reduce(out=ssum, in_=egw, op=ALU.add,
                                    axis=mybir.AxisListType.X)
            rcp = g_pool.tile([128, NST], FP, tag="rcp")
            nc.vector.reciprocal(out=rcp, in_=ssum)
            gw_all = g_pool.tile([128, NST, E], FP, tag="gw")
            nc.vector.tensor_tensor(out=gw_all, in0=egw, in1=bcast_inner(rcp, E),
                                    op=ALU.mult)
            nc.vector.tensor_add(acc_load, acc_load, gw_all)
            # one PE transpose to (st,e)-major, then contiguous DRAM bounce
            ps_gwt = tp_psum.tile([NST * E, 128], FP, tag="tp")
            nc.tensor.transpose(ps_gwt, gw_all.rearrange("p a b -> p (a b)"), ident)
            gwt_sb = g_pool.tile([NST * E, 128], FP, tag="gwt")
            nc.vector.tensor_copy(out=gwt_sb, in_=ps_gwt)
            gwt_bf = g_pool.tile([NST * E, 128], mm_dt, tag="gwtbf")
            nc.vector.tensor_copy(out=gwt_bf, in_=gwt_sb)
            gw_dram = gwd_pool.tile([E, T], FP, tag="gwd")
            nc.sync.dma_start(
                out=bass.AP(tensor=gw_dram.tensor, offset=gw_dram.offset,
                            ap=[[128, NST], [T, E], [1, 128]]),
                in_=gwt_sb,
            )
            gw_dram_bf = gwd_pool.tile([E, T], mm_dt, tag="gwdbf")
            nc.sync.dma_start(
                out=bass.AP(tensor=gw_dram_bf.tensor, offset=gw_dram_bf.offset,
                            ap=[[128, NST], [T, E], [1, 128]]),
                in_=gwt_bf,
            )
            nc.sync.dma_start(
                out=gw_outT[:, it * T:(it + 1) * T],
                in_=bass.AP(tensor=gw_dram.tensor, offset=gw_dram.offset,
                            ap=[[T, E], [1, T]]),
            )
            gw_fm = g_pool.tile([E, T], FP, tag="gwfm")
            nc.sync.dma_start(
                out=gw_fm,
                in_=bass.AP(tensor=gw_dram.tensor, offset=gw_dram.offset,
                            ap=[[T, E], [1, T]]),
            )
            gw_fm_r = g_pool.tile([E, T], FPR, tag="gwfmr")
            nc.vector.tensor_copy(out=gw_fm_r, in_=gw_fm)
            gwb = g_pool.tile([128, E, T], mm_dt, tag="gwb")
            for e in range(E):
                nc.sync.dma_start(
                    out=gwb[:, e, :],
                    in_=bass.AP(tensor=gw_dram_bf.tensor,
                                offset=gw_dram_bf.offset + e * T,
                                ap=[[0, 128], [1, T]]),
                )

            # ---- alpha head -------------------------------------------
            a1 = h_pool.tile([128, 2, T], mm_dt, tag="ha")
            for ht in range(2):
                ps_a = l1_psum.tile([128, T], FP, tag="l1")
                for th in range(2):
                    for kt in range(2):
                        nc.tensor.matmul(
                            ps_a[:, th * TH:(th + 1) * TH],
                            wa1_r[:, kt, ht * 128:(ht + 1) * 128],
                            xt_r[:, kt, th * TH:(th + 1) * TH],
                            start=(kt == 0), stop=(kt == 1),
                        )
                nc.scalar.activation(
                    out=a1[:, ht, :], in_=ps_a, func=AF.Gelu,
                    bias=ba1_sb[:, ht:ht + 1],
                )
            z_sb = out_pool.tile([C, T], FP, tag="z")
            for th in range(2):
                ps_z = l2_psum.tile([C, TH], FP, tag="l2")
                for kt in range(2):
                    nc.tensor.matmul(
                        ps_z, wa2_r[:, kt, :],
                        a1[:, kt, th * TH:(th + 1) * TH],
                        start=(kt == 0), stop=(kt == 1),
                    )
                nc.vector.tensor_scalar_add(
                    out=z_sb[:, th * TH:(th + 1) * TH], in0=ps_z,
                    scalar1=ba2_sb)
            nc.sync.dma_start(out=z_dram[:, it * T:(it + 1) * T], in_=z_sb)

            # ---- experts: h_e = gelu(x@We1[e]+be1[e]) * gw_e --------------
            he_list = []
            for e in range(E):
                he = h_pool.tile([128, 2, T], mm_dt, tag=f"he{e}")
                for ht in range(2):
                    ps_e = l1_psum.tile([128, T], FP, tag="l1")
                    for th in range(2):
                        for kt in range(2):
                            nc.tensor.matmul(
                                ps_e[:, th * TH:(th + 1) * TH],
                                we1_r[:, e, kt, ht * 128:(ht + 1) * 128],
                                xt_r[:, kt, th * TH:(th + 1) * TH],
                                start=(kt == 0), stop=(kt == 1),
                            )
                    nc.scalar.activation(
                        out=he[:, ht, :], in_=ps_e, func=AF.Gelu,
                        bias=be1_sb[:, e, ht:ht + 1],
                    )
                    nc.vector.tensor_tensor(
                        out=he[:, ht, :], in0=he[:, ht, :], in1=gwb[:, e, :],
                        op=ALU.mult,
                    )
                he_list.append(he)
            prev = (it, he_list, gw_fm_r)

        # ---- pass B: alpha = softplus(z) + 1e-6 over the flat z block ------
        spb = ctx.enter_context(tc.tile_pool(name="spb", bufs=1))
        ZF = C * N_CORE // 128          # 2560 elements per partition
        zc = spb.tile([128, ZF], FP)
        nc.sync.dma_start(
            out=zc,
            in_=bass.AP(tensor=z_dram.tensor, offset=z_dram.offset,
                        ap=[[ZF, 128], [1, ZF]]),
        )
        ec = spb.tile([128, ZF], FP)
        nc.scalar.activation(out=ec, in_=zc, func=AF.Exp)
        ac = spb.tile([128, ZF], FP)
        nc.scalar.activation(out=ac, in_=ec, func=AF.Ln, bias=1.0)
        nc.vector.tensor_scalar_add(out=ac, in0=ac, scalar1=1e-6)
        nc.sync.dma_start(
            out=bass.AP(tensor=outT.tensor, offset=outT.offset + C * N_CORE,
                        ap=[[ZF, 128], [1, ZF]]),
            in_=ac,
        )

        # ---- load: sum over token partitions via ones-matmul; host finishes
        ones_col = singles.tile([128, 1], FP)
        nc.vector.memset(ones_col, 1.0)
        ps_load = l2_psum.tile([1, NST * E], FP, tag="l2")
        nc.tensor.matmul(ps_load, ones_col,
                         acc_load.rearrange("p a b -> p (a b)"),
                         start=True, stop=True)
        load_sb = singles.tile([1, NST * E], FP)
        nc.vector.tensor_copy(out=load_sb, in_=ps_load)
        nc.sync.dma_start(out=load_out, in_=load_sb)

    nc.compile()
    return nc


_NC_CACHE = None


def kernel(**inputs):
    global _NC_CACHE
    if _NC_CACHE is None:
        if os.environ.get("BASS_MOE_FP32"):
            _NC_CACHE = build_kernel(mm_dt=FP, gate_dt=FP)
        else:
            _NC_CACHE = build_kernel()
    nc = _NC_CACHE

    f32 = lambda a: np.ascontiguousarray(np.asarray(a, dtype=np.float32))
    x = f32(inputs["x"])
    weights = {k: f32(inputs[k]) for k in
               ("Wg1", "bg1", "Wg2", "bg2", "We1", "be1", "We2", "be2",
                "Wa1", "ba1", "Wa2", "ba2")}

    in_maps = []
    for c in range(NCORES):
        m = {"x": f32(x[c * N_CORE:(c + 1) * N_CORE])}
        m.update(weights)
        in_maps.append(m)

    res = run_bass_kernel_spmd(nc, in_maps, core_ids=list(range(NCORES)),
                               trace=False)

    outT = np.concatenate([r["outT"] for r in res.results], axis=1)   # [20, N]
    gw = np.concatenate([r["gw_outT"] for r in res.results], axis=1).T  # [N, 4]
    load = np.sum([r["load_out"][0].reshape(NST, E).sum(0) for r in res.results],
                  axis=0, dtype=np.float32).astype(np.float32)
    big = np.ascontiguousarray(outT.T)
    logits = big[:, 0:C]
    alpha = big[:, C:2 * C]
    return logits, alpha, np.ascontiguousarray(gw), load
